# revision 60
# baseline (speedup 1.0000x reference)
"""DebertaV2 disentangled attention block on 8 TRN2 NeuronCores (Bass/Tile).

Head-sharded tensor parallel: 2 heads per core. Host does layout-only prep
(transpose / dtype cast); all FLOPs run on device.

v2: fp8 batched skew staging, single natural-order rel load, p2c band added
into PSUM via identity matmul, AllToAll on ctx slices + local full output
dense, per-core residual+LayerNorm on its 128 rows.
"""

import math

import numpy as np

H = 16
D = 64
HID = 1024
N = 1024
K = 1024
EPS = 1e-7
NCORES = 8
HPC = H // NCORES  # heads per core = 2
DPC = HPC * D      # head dims per core = 128
SCALE = 1.0 / math.sqrt(3.0 * D)  # applied inside exp()

W_WIN = 1151       # band window width (127 + 1024)
WPITCH = 1152      # scr row pitch (elements) per block
BLK = 128 * WPITCH # scr block size per i/j-tile
P = 128

_CACHE = {}


def _build():
    import concourse.bass as bass
    import concourse.mybir as mybir
    import concourse.tile as tile
    from concourse import bacc
    from concourse.masks import make_identity
    from contextlib import ExitStack

    f32 = mybir.dt.float32
    bf16 = mybir.dt.bfloat16
    fp8 = mybir.dt.float8e4

    nc = bacc.Bacc(None, target_bir_lowering=False, debug=False)
    names = {}

    with tile.TileContext(nc) as tc, ExitStack() as es:
        dio = es.enter_context(tc.tile_pool(name="dram_io", bufs=1, space="DRAM"))
        dwork = es.enter_context(tc.tile_pool(name="dram_work", bufs=1, space="DRAM"))

        def din(nm, shape, dt=bf16):
            t = dio.tile(shape, dt, kind="ExternalInput", name=nm, tag=nm)
            names[nm] = t.name
            return t

        hsT = din("hsT", (HID, N), fp8)       # hs[0].T
        relT = din("relT", (HID, 2 * K), fp8) # rel.T (natural order)
        wAll = din("wAll", (HID, 5 * DPC), fp8)  # [wq|wk|wv|wpk|wpq] slices
        woT = din("woT", (HID, HID), fp8)     # full Wo.T, replicated
        hs_rows = din("hs_rows", (P, HID))    # residual rows, bf16
        bq_s = din("bq_s", (DPC,), f32)
        bk_s = din("bk_s", (DPC,), f32)
        bv_s = din("bv_s", (DPC,), f32)
        bpk_s = din("bpk_s", (DPC,), f32)
        bpq_s = din("bpq_s", (DPC,), f32)
        bo_t = din("bo", (HID,), f32)
        lng_t = din("ln_g", (HID,), f32)
        lnb_t = din("ln_b", (HID,), f32)

        out_t = dio.tile((P, HID), bf16, kind="ExternalOutput", name="out", tag="out")
        names["out"] = out_t.name

        a2a_in = [dwork.tile((8 * 64, P), fp8, name=f"a2a_in{h_}",
                              tag=f"a2a_in{h_}") for h_ in range(HPC)]
        a2a_out = [dwork.tile((8 * 64, P), fp8, name=f"a2a_out{h_}",
                              tag=f"a2a_out{h_}") for h_ in range(HPC)]

        # ---- SBUF pools -------------------------------------------------
        wt = es.enter_context(tc.tile_pool(name="wt", bufs=1))
        work = es.enter_context(tc.tile_pool(name="work", bufs=1))
        psA = es.enter_context(tc.tile_pool(name="psA", bufs=3, space="PSUM"))

        Iden = mybir.ActivationFunctionType.Identity
        Exp = mybir.ActivationFunctionType.Exp
        Sqrt = mybir.ActivationFunctionType.Sqrt
        ADD = mybir.AluOpType.add
        MUL = mybir.AluOpType.mult
        SUB = mybir.AluOpType.subtract
        BYPASS = mybir.AluOpType.bypass

        # ---- load order: hsT + weights first so PE starts ASAP -----------
        # k-tiles are loaded in PAIRS [128, 2, width] so contractions can
        # use fp8 DoubleRow matmuls (2 k-tiles per instruction).
        DR = mybir.MatmulPerfMode.DoubleRow

        def load_pairs(src, width, nm, persist=False):
            tiles, frees = [], []
            for t2 in range(4):
                if persist:
                    a, fa = wt.tile([P, 2 * width], fp8, name=f"{nm}{t2}",
                                    tag=f"{nm}{t2}"), None
                else:
                    a, fa = tc.tile([P, 2 * width], fp8, name=f"{nm}{t2}")
                ap = a[:]
                nc.sync.dma_start(
                    bass.AP(ap.tensor, ap.offset,
                            [list(ap.ap[0]), [width, 2], [1, width]]),
                    bass.AP(src[:].tensor,
                            src[:].offset + 256 * t2 * width,
                            [[width, P], [128 * width, 2], [1, width]]))
                tiles.append(a)
                frees.append(fa)
            return tiles, frees

        def pair_slice(tile_, pair_stride, c0, w):
            ap = tile_[:]
            return bass.AP(ap.tensor, ap.offset + c0,
                           [list(ap.ap[0]), [pair_stride, 2], [1, w]])

        def bias_tile(nm, src, n=DPC):
            t = wt.tile([n, 1], f32, name=nm, tag=nm)
            nc.sync.dma_start(t[:], bass.AP(src[:].tensor, src[:].offset, [[1, n]]))
            return t

        # prefetch the Iden/Exp activation table while the first loads run
        warm = wt.tile([1, 1], f32, name="warm", tag="warm")
        nc.vector.memset(warm[:], 0.0)
        warm2 = wt.tile([1, 1], f32, name="warm2", tag="warm2")
        nc.scalar.activation(warm2[:], warm[:], Iden)

        hsP_sb, hsP_free = load_pairs(hsT, N, "hsP")
        wP_sb, _ = load_pairs(wAll, 5 * DPC, "wP", persist=True)
        bq_sb = bias_tile("bq_sb", bq_s)
        bk_sb = bias_tile("bk_sb", bk_s)
        ident = wt.tile([P, P], bf16, name="ident", tag="ident")
        make_identity(nc, ident[:])
        ident8 = wt.tile([P, P], fp8, name="ident8", tag="ident8")
        nc.vector.tensor_copy(ident8[:], ident[:])

        # ---- projections -------------------------------------------------
        qT = wt.tile([P, N], bf16, name="qT", tag="qT")
        kT = wt.tile([P, N], bf16, name="kT", tag="kT")
        pkT = wt.tile([P, 2 * K], bf16, name="pkT", tag="pkT")
        pqT = wt.tile([P, 2 * K], bf16, name="pqT", tag="pqT")

        def project(dst, wblk, rhs_pairs, width, bias):
            for c0 in range(0, width, 512):
                ps = psA.tile([P, 512], f32, name="pp", tag="pp")
                for t2 in range(4):
                    nc.tensor.matmul(
                        ps[:], pair_slice(wP_sb[t2], 5 * DPC, DPC * wblk, DPC),
                        pair_slice(rhs_pairs[t2], width, c0, 512),
                        start=(t2 == 0), stop=(t2 == 3), perf_mode=DR)
                nc.scalar.activation(dst[:, c0:c0 + 512], ps[:], Iden,
                                     bias=bias[:])

        bpk_sb = bias_tile("bpk_sb", bpk_s)
        bpq_sb = bias_tile("bpq_sb", bpq_s)
        relP_sb, relP_free = load_pairs(relT, 2 * K, "relP")

        project(qT, 0, hsP_sb, N, bq_sb)
        project(kT, 1, hsP_sb, N, bk_sb)
        project(pkT, 3, relP_sb, 2 * K, bpk_sb)

        # v in [j, d] layout + 1/16 column per head: va[jt] is [128, 132]
        va = []

        def emit_va():
            for jt in range(8):
                t = wt.tile([P, 132], bf16, name=f"va{jt}", tag=f"va{jt}")
                ps = psA.tile([P, DPC], f32, name="pv", tag="pp")
                for t2 in range(4):
                    nc.tensor.matmul(
                        ps[:], pair_slice(hsP_sb[t2], N, 128 * jt, 128),
                        pair_slice(wP_sb[t2], 5 * DPC, DPC * 2, DPC),
                        start=(t2 == 0), stop=(t2 == 3), perf_mode=DR)
                nc.scalar.copy(t[:, 0:64], ps[:, 0:64])
                nc.scalar.copy(t[:, 66:130], ps[:, 64:128])
                nc.vector.memset(t[:, 64:65], 1.0 / 16.0)
                nc.vector.memset(t[:, 130:131], 1.0 / 16.0)
                va.append(t)
            for f in reversed(hsP_free):
                f()
        # per-head [128, 64] broadcast of 16*bv (16x scale into fp8 ctx;
        # compensated by 1/16 after the out dense)
        bv16bc = []
        for h_ in range(HPC):
            t = wt.tile([P, 64], f32, name=f"bv16bc{h_}", tag=f"bv16bc{h_}")
            nc.sync.dma_start(t[:], bass.AP(bv_s[:].tensor,
                                            bv_s[:].offset + 64 * h_,
                                            [[0, P], [1, 64]]))
            nc.vector.tensor_scalar_mul(t[:], t[:], 16.0)
            bv16bc.append(t)

        # ---- banded position scores: batched fp8 skew staging ------------
        # One (head, dir) group: 8 band blocks [128, 1151] -> blkbuf fp8 ->
        # one DMA to scr -> one skewed gather back as [128, 8*1024] fp8.
        cp_idx = [0]
        cp_rot = {0: None, 1: None}

        def band_group(lhs, src_T, hd, reverse, tagp, rot):
            # blkbuf row p holds the 8 band blocks back-to-back at pitch
            # WPITCH; scr mirrors it partition-major so both the store and
            # the skewed gather are 128 long descriptors.
            blkbuf = work.tile([P, 8 * WPITCH], fp8, name=f"blk_{tagp}",
                               tag="blkbuf", bufs=2)
            bap = blkbuf[:]
            # init the 8 pad columns (col 1151 of each block)
            nc.vector.memset(
                bass.AP(bap.tensor, bap.offset + W_WIN,
                        [list(bap.ap[0]), [WPITCH, 8]]), 0.0)
            for r in range(8):
                w0 = (1 + 128 * r) if reverse else (897 - 128 * r)
                for (c0, w) in ((0, 512), (512, 512), (1024, 127)):
                    ps = psA.tile([P, 512], f32, name="pblk", tag="pp")
                    nc.tensor.matmul(
                        ps[:, 0:w],
                        lhs[hd, 128 * r:128 * (r + 1)],
                        src_T[hd, w0 + c0:w0 + c0 + w],
                        start=True, stop=True)
                    if reverse:
                        dst = bass.AP(bap.tensor,
                                      bap.offset + WPITCH * r + 1150 - c0,
                                      [list(bap.ap[0]), [-1, w]])
                    else:
                        dst = blkbuf[:, WPITCH * r + c0:WPITCH * r + c0 + w]
                    eng = rot[cp_idx[0] % len(rot)]
                    cp_idx[0] += 1
                    if eng is nc.scalar:
                        eng.copy(dst, ps[:, 0:w])
                    else:
                        eng.tensor_copy(dst, ps[:, 0:w])
            scr = dwork.tile((P * 8 * WPITCH,), fp8, name=f"scr_{tagp}",
                             tag="scr", bufs=2)
            h = scr[:].tensor
            o = scr[:].offset
            # g[p, WPITCH*r + x] = blk_r[p, 127 - p + x]; store + gather in
            # two halves so blocks 0-3 become consumable while 4-7 compute.
            # Consumers read at most col WPITCH*r + 1152 of g, so the first
            # gather stops at 4*WPITCH - 127 and stays inside half 0.
            HW2 = 4 * WPITCH
            g = work.tile([P, 8 * WPITCH], fp8, name=f"g_{tagp}",
                          tag=f"g{tagp[0]}", bufs=2)
            nc.sync.dma_start(
                bass.AP(h, o, [[8 * WPITCH, P], [1, HW2]]),
                blkbuf[:, 0:HW2])
            nc.sync.dma_start(
                g[:, 0:HW2 - 127],
                bass.AP(h, o + 127, [[8 * WPITCH - 1, P], [1, HW2 - 127]]))
            nc.sync.dma_start(
                bass.AP(h, o + HW2, [[8 * WPITCH, P], [1, HW2]]),
                blkbuf[:, HW2:2 * HW2])
            nc.sync.dma_start(
                g[:, HW2:2 * HW2],
                bass.AP(h, o + 127 + HW2, [[8 * WPITCH - 1, P], [1, HW2]]))
            return g

        # ---- attention per head -----------------------------------------
        ctxT = wt.tile([P, N], fp8, name="ctxT", tag="ctxT")
        ctx_all = [wt.tile([64, N], fp8, name=f"ctx_all{h_}",
                           tag=f"ctx_all{h_}") for h_ in range(HPC)]
        deferred = {}

        def emit_deferred():
            # per-head per-pair woT slices [64, 2, HID]: partition d holds
            # Wo.T row 256*s2 + 128*i + 64*h + d
            woT_sb = {}
            for h_ in range(HPC):
                for s2 in range(4):
                    t = wt.tile([64, 2 * HID], fp8, name=f"woP{h_}{s2}",
                                tag=f"woP{h_}{s2}")
                    ap = t[:]
                    nc.sync.dma_start(
                        bass.AP(ap.tensor, ap.offset,
                                [list(ap.ap[0]), [HID, 2], [1, HID]]),
                        bass.AP(woT[:].tensor,
                                woT[:].offset + (256 * s2 + 64 * h_) * HID,
                                [[HID, 64], [128 * HID, 2], [1, HID]]))
                    woT_sb[(h_, s2)] = t

            def bcast_tile(nm, src, dt=f32):
                t = wt.tile([P, HID], dt, name=nm, tag=nm)
                eng = nc.sync if dt == f32 else nc.gpsimd
                eng.dma_start(t[:], bass.AP(src[:].tensor, src[:].offset,
                                            [[0, P], [1, HID]]))
                return t

            bo_bc = bcast_tile("bo_bc", bo_t)
            g_bc = bcast_tile("g_bc", lng_t, bf16)
            b_bc = bcast_tile("b_bc", lnb_t, bf16)
            hsr_sb = wt.tile([P, HID], bf16, name="hsr_sb", tag="hsr_sb")
            nc.sync.dma_start(hsr_sb[:], hs_rows[:])
            # residual + out-dense bias, precomputed off the critical tail
            hb = wt.tile([P, HID], f32, name="hb", tag="hb")
            nc.vector.tensor_add(hb[:], hsr_sb[:], bo_bc[:])
            deferred.update(woT_sb=woT_sb, g_bc=g_bc, b_bc=b_bc, hb=hb)

        # band groups emitted interleaved with the remaining projections so
        # the store+gather DMA chain starts as early as possible; both
        # heads' bands precede the score phases so PE fills the latency.
        rot0 = [nc.vector, nc.scalar, nc.vector]
        rot1 = [nc.vector, nc.scalar, nc.vector]
        hd0 = slice(0, 64)
        hd1 = slice(64, 128)
        g_c0 = band_group(qT, pkT, hd0, True, "c0", rot0)
        project(pqT, 4, relP_sb, 2 * K, bpq_sb)
        for f in reversed(relP_free):
            f()
        g_p0 = band_group(kT, pqT, hd0, False, "p0", rot0)
        g_c1 = band_group(qT, pkT, hd1, True, "c1", rot1)
        g_p1 = band_group(kT, pqT, hd1, False, "p1", rot1)
        bands = {0: (g_c0, g_p0), 1: (g_c1, g_p1)}
        emit_va()
        emit_deferred()

        for h in range(HPC):
            hd = slice(64 * h, 64 * h + 64)
            g_c, g_p = bands[h]

            e_all = []
            for jt in range(8):
                e = work.tile([P, N], bf16, name=f"expST{jt}", tag="expST",
                              bufs=17)
                for c in range(2):
                    st = psA.tile([P, 512], f32, name="st", tag="st", bufs=2)
                    nc.tensor.matmul(st[:], kT[hd, 128 * jt:128 * (jt + 1)],
                                     qT[hd, 512 * c:512 * (c + 1)],
                                     start=True, stop=False)
                    for rr in range(4):
                        r = 4 * c + rr
                        nc.tensor.matmul(st[:, 128 * rr:128 * (rr + 1)],
                                         g_c[:, WPITCH * r + 128 * jt:
                                             WPITCH * r + 128 * (jt + 1)],
                                         ident8[:], start=False, stop=False)
                    nc.tensor.matmul(st[:], ident8[:],
                                     g_p[:, WPITCH * jt + 512 * c:
                                         WPITCH * jt + 512 * (c + 1)],
                                     start=False, stop=True)
                    nc.scalar.activation(e[:, 512 * c:512 * (c + 1)], st[:],
                                         Exp, scale=SCALE)
                e_all.append(e)

            # attn@v per i-tile: out[i, d] with the 1/16-scaled denom in
            # col 64, so 1/denom lands as a per-partition scalar (no
            # broadcast roundtrip); normalize, then transpose to [d, i].
            ctxR = work.tile([P, 8 * 64], bf16, name=f"ctxR{h}", tag="ctxR",
                             bufs=2)
            for it in range(8):
                att = psA.tile([P, 65], f32, name="att", tag="att", bufs=2)
                for jt in range(8):
                    nc.tensor.matmul(att[:],
                                     e_all[jt][:, 128 * it:128 * (it + 1)],
                                     va[jt][:, 66 * h:66 * h + 65],
                                     start=(jt == 0), stop=(jt == 7))
                r16 = work.tile([P, 1], f32, name="r16", tag="r16", bufs=4)
                nc.vector.reciprocal(r16[:], att[:, 64:65])
                csl = ctxR[:, 64 * it:64 * (it + 1)]
                nc.vector.scalar_tensor_tensor(csl, att[:, 0:64], r16[:],
                                               bv16bc[h][:], op0=MUL, op1=ADD)
                psT = psA.tile([64, P], bf16, name="psT", tag="psT", bufs=1)
                nc.tensor.transpose(psT[:], csl, ident[:])
                eng = (nc.vector, nc.scalar)[cp_idx[0] % 2]
                cp_idx[0] += 1
                dstc = ctxT[64 * h:64 * (h + 1), 128 * it:128 * (it + 1)]
                if eng is nc.scalar:
                    eng.copy(dstc, psT[:])
                else:
                    eng.tensor_copy(dstc, psT[:])

            # per-head AllToAll of this head's ctx slices: overlaps the
            # other head's compute. a2a_in[h][(s*64 + d), c] = ctxT[64h+d,
            # 128s + c]
            aap = a2a_in[h][:]
            nc.sync.dma_start(
                bass.AP(aap.tensor, aap.offset,
                        [[P, 64], [64 * P, 8], [1, P]]),
                ctxT[64 * h:64 * (h + 1), :])
            nc.gpsimd.collective_compute(
                "AllToAll", BYPASS, replica_groups=[list(range(NCORES))],
                ins=[a2a_in[h][:]], outs=[a2a_out[h][:]])
            oap = a2a_out[h][:]
            nc.sync.dma_start(
                ctx_all[h][:],
                bass.AP(oap.tensor, oap.offset,
                        [[P, 64], [64 * P, 8], [1, P]]))

        # ---- output dense (own 128 rows) + residual + LayerNorm ----------
        woT_sb = deferred["woT_sb"]
        g_bc = deferred["g_bc"]
        b_bc = deferred["b_bc"]
        hb = deferred["hb"]
        # h0's dense half runs while h1's collective is still in flight
        x = wt.tile([P, HID], bf16, name="x", tag="x")
        x0 = wt.tile([P, HID], f32, name="x0", tag="x0")
        for h_ in range(HPC):
            for c in range(2):
                po = psA.tile([P, 512], f32, name="po", tag="pp")
                cap = ctx_all[h_][:]
                for s2 in range(4):
                    wb = woT_sb[(h_, s2)][:]
                    nc.tensor.matmul(
                        po[:],
                        bass.AP(cap.tensor, cap.offset + 256 * s2,
                                [list(cap.ap[0]), [128, 2], [1, 128]]),
                        bass.AP(wb.tensor, wb.offset + 512 * c,
                                [list(wb.ap[0]), [HID, 2], [1, 512]]),
                        start=(s2 == 0), stop=(s2 == 3), perf_mode=DR)
                if h_ == 0:
                    nc.vector.scalar_tensor_tensor(
                        x0[:, 512 * c:512 * (c + 1)], po[:], 1.0 / 16.0,
                        hb[:, 512 * c:512 * (c + 1)], op0=MUL, op1=ADD)
                else:
                    nc.vector.scalar_tensor_tensor(
                        x[:, 512 * c:512 * (c + 1)], po[:], 1.0 / 16.0,
                        x0[:, 512 * c:512 * (c + 1)], op0=MUL, op1=ADD)

        stats = wt.tile([P, 2, 6], f32, name="stats", tag="stats")
        mv = wt.tile([P, 2], f32, name="mv", tag="mv")
        for s in range(2):
            nc.vector.bn_stats(stats[:, s, :], x[:, 512 * s:512 * (s + 1)])
        nc.vector.bn_aggr(mv[:], stats[:])
        epsb = wt.tile([P, 1], f32, name="epsb", tag="epsb")
        nc.vector.memset(epsb[:], EPS)
        std = wt.tile([P, 1], f32, name="std", tag="std")
        nc.scalar.activation(std[:], mv[:, 1:2], Sqrt, bias=epsb[:])
        rstd = wt.tile([P, 1], f32, name="rstd", tag="rstd")
        nc.vector.reciprocal(rstd[:], std[:])

        t1 = wt.tile([P, HID], bf16, name="t1", tag="t1")
        nc.vector.scalar_tensor_tensor(t1[:], x[:], mv[:, 0:1], g_bc[:],
                                       op0=SUB, op1=MUL)
        yout = wt.tile([P, HID], bf16, name="yout", tag="yout")
        nc.vector.scalar_tensor_tensor(yout[:], t1[:], rstd[:], b_bc[:],
                                       op0=MUL, op1=ADD)
        nc.sync.dma_start(out_t[:], yout[:])

    nc.compile()
    return nc, names


def _get_compiled():
    if "nc" not in _CACHE:
        nc, names = _build()
        _CACHE["nc"] = nc
        _CACHE["names"] = names
    return _CACHE["nc"], _CACHE["names"]


def _prep_in_maps(inputs):
    import ml_dtypes

    bf = ml_dtypes.bfloat16
    f8 = ml_dtypes.float8_e4m3
    hs = np.asarray(inputs["hidden_states"], np.float32)[0]      # (N, HID)
    rel = np.asarray(inputs["rel_embeddings"], np.float32)       # (2K, HID)
    hsT = np.ascontiguousarray(hs.T).astype(f8)
    relT = np.ascontiguousarray(rel.T).astype(f8)
    woT = np.ascontiguousarray(np.asarray(inputs["Wo"], np.float32).T).astype(f8)

    def wT(w, r):
        w = np.asarray(w, np.float32)
        return np.ascontiguousarray(w[DPC * r:DPC * (r + 1), :].T)

    in_maps = []
    for r in range(NCORES):
        wall = np.concatenate(
            [wT(inputs[k], r) for k in ("Wq", "Wk", "Wv", "Wpk", "Wpq")],
            axis=1).astype(f8)
        m = {
            "hsT": hsT,
            "relT": relT,
            "wAll": np.ascontiguousarray(wall),
            "woT": woT,
            "hs_rows": np.ascontiguousarray(hs[P * r:P * (r + 1), :]).astype(bf),
            "bq_s": np.asarray(inputs["bq"], np.float32)[DPC * r:DPC * (r + 1)],
            "bk_s": np.asarray(inputs["bk"], np.float32)[DPC * r:DPC * (r + 1)],
            "bv_s": np.asarray(inputs["bv"], np.float32)[DPC * r:DPC * (r + 1)],
            "bpk_s": np.asarray(inputs["bpk"], np.float32)[DPC * r:DPC * (r + 1)],
            "bpq_s": np.asarray(inputs["bpq"], np.float32)[DPC * r:DPC * (r + 1)],
            "bo": np.asarray(inputs["bo"], np.float32),
            "ln_g": np.asarray(inputs["ln_g"], np.float32),
            "ln_b": np.asarray(inputs["ln_b"], np.float32),
        }
        in_maps.append(m)
    return in_maps


def run(inputs, trace=False):
    from concourse.bass_utils import run_bass_kernel_spmd

    nc, names = _get_compiled()
    logical = _prep_in_maps(inputs)
    in_maps = [{names[k]: v for k, v in m.items()} for m in logical]
    res = run_bass_kernel_spmd(nc, in_maps, list(range(NCORES)), trace=trace)
    outs = [res.results[r][names["out"]].astype(np.float32) for r in range(NCORES)]
    full = np.concatenate(outs, axis=0).reshape(1, N, HID)
    return full, res


def kernel(**inputs) -> np.ndarray:
    full, _ = run(inputs, trace=False)
    return full
